# revision 5
# baseline (speedup 1.0000x reference)
"""GAT layer kernel for Trainium2, 8-core data-parallel over batch.

Math (per batch b, head h):
    h = x @ W                              [N, H*HD]
    s_n = <h[n, h*HD:(h+1)*HD], a_src[h]>  t_n likewise with a_dst
    A[j, i] = exp(leakyrelu(s_i + t_j, 0.2))
    out[i]  = (sum_j A[j, i] * h_j) / (sum_j A[j, i])

Key identity: softmax columns are scale-invariant, so drop the e^{s_i}
column factor entirely:
    A'[j, i] = A[j, i] * e^{-s_i} = max(e^{t_j}, e^{0.2 t_j} * u_i),
    u_i = e^{-0.8 s_i}
Both numerator and Z pick up the same e^{-s_i}, which cancels in the
division. Each [128, N] attention tile is then ONE vector tensor_scalar
op (in0 = broadcast u row, two per-partition scalar columns e^{t_j} /
e^{0.2 t_j}, ops mult+max).

Per core (= one batch element):
  - xT and [W | Wa] ship as fp16 (host precomputes Wa = W @ a_ext and
    casts; |x|<6 and |Wa|<3 sit well inside f16 range): half the HBM
    bytes on the longest load, zero on-device casts, full 16-bit PE rate.
  - PE warm-up: the PE's HAM clock gate defaults to 1.2 GHz and only
    releases to 2.4 GHz after ~3.4us of sustained matmul activity. The
    initial xT DMA wait (~2.5us) is dead PE time, so we burn it on K=1
    junk matmuls into the ub_ps scratch PSUM -- by the time real
    matmuls start the PE is at (or near) full clock, instead of
    flipping warm 9us into the kernel.
  - st rows = Wa^T @ xT via one small fp16 matmul per column half;
    u = Exp(-0.8 s) on ACT.
  - u broadcast [1,N] -> [128,N]: head 0 via a K=1 ones-column outer
    product on the PE (+ copies: c0 on DVE, c1 on ACT); heads 1-3 via
    DRAM round-trip broadcast DMAs, split per column half and issued as
    soon as the matching uexp half lands.
  - h_node blocks AND transposed t columns from the SAME per-j-tile
    matmul with rhs [W | Wa] (136 fp16 cols; PSUM pitch bank-aligned at
    256 f32). PE order: st c0, hnst jt0-3, st c1, hnst jt4-7, then the
    two u outer products -- projections never wait on the u chain.
    The e^{t}/e^{0.2 t} scalar columns and bf16 weight tiles are carved
    on ACT in chunks ordered so head 0's first tiles (jt0+jt1, which
    its interleaved column-half loop consumes together) gate on the
    FIRST wt copy, not the bulk one.
  - main loop per (head, j-tile): one tensor_scalar -> A' tile (bf16),
    two 512-col matmuls accumulate [h_node | ones]^T @ A' into PSUM
    [33, N] (row 32 = Z). The PE pipelines at ~216 ns/512 cols and the
    DVE produces one A' tile per ~456 ns; they are closely matched.
  - per-head epilogue on the otherwise-idle ACT engine, pipelined with
    later heads' bulk work: nrz row = 1/Z via Ln then Exp(-1 * .)
    (the direct Reciprocal activation is gated off in bass for accuracy;
    ln/exp tables live in one act-table set so there is a single table
    load), plus the numerator copy PSUM->SBUF which frees the PSUM
    banks for the rzb broadcast matmuls. Head 3's epilogue is the tail:
    its num copy goes per column half on the then-idle DVE while ACT
    runs the ln/exp chain, so the two critical paths overlap.
  - tail per column half: one K=97 bf16 indicator matmul broadcasts the
    1/Z rows to all 128 (h,d) output rows, one fused num * rzb multiply,
    bf16 DMA out on separate queues.
  - host transposes out^T back to node-major (and bf16 -> f32) when
    unsharding.
"""

import numpy as np

B, N, IN_F, OUT_F, H = 8, 1024, 128, 128, 4
HD = OUT_F // H  # 32
NEG = 0.2
N_CORES = 8
NT = N // 128  # 8 node tiles
CW = 256  # hnst per-j-tile column pitch (= padded rhs width)

A_DT = "bfloat16"  # dtype of the attention tiles + matmul weights
N_WARM = 7  # K=1 junk matmuls (512 cols each) to lift the HAM clock gate

_CACHE = {}


def _build_nc():
    import concourse.bacc as bacc
    import concourse.tile as tile
    from concourse import mybir

    f32 = mybir.dt.float32
    f16 = mybir.dt.float16
    adt = getattr(mybir.dt, A_DT)
    AF = mybir.ActivationFunctionType
    ALU = mybir.AluOpType

    nc = bacc.Bacc("TRN2", target_bir_lowering=False, debug=False,
                   num_devices=N_CORES)

    xT = nc.declare_dram_parameter("xT", [IN_F, N], f16, isOutput=False)
    WWa_d = nc.declare_dram_parameter("WWa", [IN_F, OUT_F + 2 * H], f16,
                                      isOutput=False)
    ind97_d = nc.declare_dram_parameter("ind97", [3 * HD + 1, OUT_F], adt,
                                        isOutput=False)
    outT = nc.declare_dram_parameter("outT", [OUT_F, N], adt, isOutput=True)

    u_dram = nc.dram_tensor("u_scratch", [H, N], adt)

    with tile.TileContext(nc) as tc:
      with (
        tc.tile_pool(name="const", bufs=1) as cpool,
        tc.tile_pool(name="atile", bufs=12) as apool,
        tc.tile_pool(name="tail", bufs=1) as tpool,
      ):
        with (
            tc.tile_pool(name="ps_pre", bufs=1, space="PSUM") as pspre,
            tc.tile_pool(name="ps_ub", bufs=1, space="PSUM") as psub,
        ):
            # ---- load inputs: [W | Wa] ships pre-cast to fp16 (the
            # host cast rounds identically to a device cast), ahead of
            # the xT halves on its queue; second xT half via the Scalar
            # engine's queue ----
            xT_sb = cpool.tile([IN_F, N], f16, tag="xT")
            WW16 = cpool.tile([IN_F, OUT_F + 2 * H], f16, tag="WW16")
            nc.sync.dma_start(out=WW16, in_=WWa_d[:])
            nc.sync.dma_start(out=xT_sb[:, 0:512], in_=xT[:, 0:512])
            nc.scalar.dma_start(out=xT_sb[:, 512:N], in_=xT[:, 512:N])
            # ind97 ships pre-cast as bf16 (no on-device cast in the DVE
            # FIFO gating the ramp)
            ind97 = cpool.tile([3 * HD + 1, OUT_F], adt, tag="ind97")
            nc.gpsimd.dma_start(out=ind97, in_=ind97_d[:])

            # ones row for the K=1 u-broadcast outer product + a junk
            # rhs row for the HAM warm-up matmuls
            ones1 = cpool.tile([1, 128], adt, tag="ones1")
            nc.gpsimd.memset(ones1, 1.0)
            wsrc = cpool.tile([1, 512], adt, tag="wsrc")
            nc.gpsimd.memset(wsrc, 1.0)

            War16 = WW16[:, OUT_F:OUT_F + H]

            st_ps = pspre.tile([H, N], f32, tag="st")
            u_rows = cpool.tile([H, N], adt, tag="u_rows")
            hnst = pspre.tile([128, NT * CW], f32, tag="hnst")
            hnst_v = hnst[:].rearrange("p (jt c) -> p jt c", c=CW)
            ub_ps = psub.tile([128, N], f32, tag="ub0ps")
            u_b0 = cpool.tile([128, N], adt, tag="ub0")
            u_b = [u_b0]
            for h in range(1, H):
                u_b.append(cpool.tile([128, N], adt, name=f"ub{h}",
                                      tag=f"ub{h}"))

            etc = cpool.tile([128, H * NT], f32, tag="etc")
            etc02 = cpool.tile([128, H * NT], f32, tag="etc02")
            tcols = hnst_v[:, :, OUT_F + H:OUT_F + 2 * H]
            etc_v = etc[:].rearrange("p (h jt) -> p jt h", jt=NT)
            etc02_v = etc02[:].rearrange("p (h jt) -> p jt h", jt=NT)
            wt_all = cpool.tile([128, NT * 33 * H], adt, tag="wt")
            wt_v = wt_all[:].rearrange("p (jt h c) -> p jt h c", h=H, c=33)
            nc.gpsimd.memset(wt_v[:, :, :, 32:33], 1.0)
            hn_v = hnst_v[:, :, 0:OUT_F].rearrange(
                "p jt (h d) -> p jt h d", d=HD)

            # tail tensors: nrz rows land on partitions 0/32/64/96 (the
            # only offsets engines can address); dead lanes memset 0 so
            # the K=97 broadcast matmul sees finite values under weight 0
            NP = 3 * HD + 1
            num4 = tpool.tile([128, N], f32, tag="num4")
            lnz = tpool.tile([NP, N], f32, tag="lnz")
            nrz = tpool.tile([NP, N], adt, tag="nrz")
            nc.gpsimd.memset(nrz, 0.0)

            # ---- HAM warm-up: junk K=1 matmuls into the ub_ps scratch
            # (overwritten later by the real outer products) keep the PE
            # busy through the initial DMA wait so the clock gate lifts
            # to 2.4 GHz before real work arrives ----
            for _ in range(N_WARM):
                nc.tensor.matmul(ub_ps[:, 0:512], ones1, wsrc,
                                 start=True, stop=True)

            # ---- ramp, fully 512-column-chunked: per half c the chain
            # st matmul -> uexp -> u broadcasts starts as soon as that
            # half of xT has landed; hnst projections for the j-tiles
            # that half covers go immediately after on the PE ----
            bcast_eng = {1: nc.sync, 2: nc.scalar, 3: nc.gpsimd}

            def half_chain(c):
                cs = slice(512 * c, 512 * (c + 1))
                nc.tensor.matmul(st_ps[:, cs], War16, xT_sb[:, cs],
                                 start=True, stop=True)
                nc.scalar.activation(out=u_rows[:, cs], in_=st_ps[:, cs],
                                     func=AF.Exp, scale=-0.8)
                for jt in (0, 1, 2, 3) if c == 0 else (4, 5, 6, 7):
                    nc.tensor.matmul(
                        hnst[:, CW * jt:CW * jt + OUT_F + 2 * H],
                        xT_sb[:, 128 * jt:128 * (jt + 1)],
                        WW16, start=True, stop=True)
                # heads 1-3 u broadcast via DRAM round trip, per column
                # half so each half's broadcasts start as soon as its
                # uexp lands (SBUF sources cannot carry the stride-0
                # partition AP a direct broadcast would need)
                nc.sync.dma_start(out=u_dram[:, cs], in_=u_rows[:, cs])
                for h in range(1, H):
                    bcast_eng[h].dma_start(
                        out=u_b[h][:, cs],
                        in_=u_dram[h:h + 1, cs].to_broadcast([128, 512]))

            half_chain(0)
            # t scalar columns for jt0-1 slot between the two uexps
            nc.scalar.activation(out=etc_v[:, 0:2], in_=tcols[:, 0:2],
                                 func=AF.Exp)
            nc.scalar.activation(out=etc02_v[:, 0:2], in_=tcols[:, 0:2],
                                 func=AF.Exp, scale=NEG)
            half_chain(1)
            # weight tiles jt0-1 first: head 0's interleaved column-half
            # loop consumes jt0 AND jt1 together, so its first matmul
            # gates on this small copy instead of the jt2-7 bulk one
            nc.scalar.copy(out=wt_v[:, 0:2, :, 0:32], in_=hn_v[:, 0:2])
            # head-0 u broadcast: K=1 outer product per half; c0 copy on
            # DVE (first thing in its FIFO), c1 copy on ACT so the DVE
            # can start head 0's first A' tiles without waiting
            nc.tensor.matmul(ub_ps[:, 0:512], ones1, u_rows[0:1, 0:512],
                             start=True, stop=True)
            nc.tensor.matmul(ub_ps[:, 512:N], ones1, u_rows[0:1, 512:N],
                             start=True, stop=True)
            nc.vector.tensor_copy(out=u_b0[:, 0:512], in_=ub_ps[:, 0:512])
            nc.scalar.copy(out=u_b0[:, 512:N], in_=ub_ps[:, 512:N])
            nc.scalar.activation(out=etc_v[:, 2:], in_=tcols[:, 2:],
                                 func=AF.Exp)
            nc.scalar.activation(out=etc02_v[:, 2:], in_=tcols[:, 2:],
                                 func=AF.Exp, scale=NEG)
            nc.scalar.copy(out=wt_v[:, 2:, :, 0:32], in_=hn_v[:, 2:])
            wts = [wt_all[:, 132 * jt:132 * (jt + 1)] for jt in range(NT)]

        # ---- main loop: one tensor_scalar + two matmuls per (h, jt) ----
        with tc.tile_pool(name="ps_main", bufs=4, space="PSUM") as psmain:
            for h in range(H):
                oh = psmain.tile([33, N], f32, tag="oh")
                if h == 0:
                    # head 0's jt0/jt1 go in column halves ordered so
                    # their c0 work streams while c1's u row is still
                    # being copied out of the outer product; the
                    # interleaved matmuls hit disjoint PSUM banks, so
                    # accumulation groups stay well-formed per half
                    ats = [apool.tile([128, N], adt, name=f"a0{k}",
                                      tag="at") for k in range(2)]
                    for c in range(2):
                        cs = slice(512 * c, 512 * (c + 1))
                        for k in range(2):
                            nc.vector.tensor_scalar(
                                out=ats[k][:, cs], in0=u_b[0][:, cs],
                                scalar1=etc02[:, k:k + 1],
                                scalar2=etc[:, k:k + 1],
                                op0=ALU.mult, op1=ALU.max)
                        for k in range(2):
                            nc.tensor.matmul(
                                oh[:, cs], wts[k][:, 0:33], ats[k][:, cs],
                                start=(k == 0), stop=False)
                    jts = range(2, NT)
                else:
                    jts = range(NT)
                for jt in jts:
                    idx = h * NT + jt
                    a_t = apool.tile([128, N], adt, tag="at")
                    nc.vector.tensor_scalar(
                        out=a_t, in0=u_b[h],
                        scalar1=etc02[:, idx:idx + 1],
                        scalar2=etc[:, idx:idx + 1],
                        op0=ALU.mult, op1=ALU.max)
                    for c in range(2):
                        nc.tensor.matmul(
                            oh[:, 512 * c:512 * (c + 1)],
                            wts[jt][:, 33 * h:33 * (h + 1)],
                            a_t[:, 512 * c:512 * (c + 1)],
                            start=(jt == 0), stop=(jt == NT - 1))
                # per-head epilogue, pipelined with the next heads' bulk
                # work on the otherwise-idle ACT engine: 1/Z via Ln +
                # Exp(-1 * .) per column half (each half's chain starts
                # as soon as that half's accumulation group closes),
                # then the numerator rows PSUM->SBUF (frees the banks
                # for the rzb matmuls). Head 3's num copy runs on the
                # then-idle DVE instead, per half, overlapping ACT's
                # ln/exp chain -- both tail-critical paths in parallel.
                for c in range(2):
                    cs = slice(512 * c, 512 * (c + 1))
                    nc.scalar.activation(out=lnz[HD * h:HD * h + 1, cs],
                                         in_=oh[32:33, cs], func=AF.Ln)
                    nc.scalar.activation(out=nrz[HD * h:HD * h + 1, cs],
                                         in_=lnz[HD * h:HD * h + 1, cs],
                                         func=AF.Exp, scale=-1.0)
                if h == 3:
                    for c in range(2):
                        cs = slice(512 * c, 512 * (c + 1))
                        nc.vector.tensor_copy(
                            out=num4[HD * h:HD * (h + 1), cs],
                            in_=oh[0:32, cs])
                else:
                    nc.scalar.copy(out=num4[HD * h:HD * (h + 1), :],
                                   in_=oh[0:32, :])

        # ---- tail: rzb[32h+d, i'] = nrz[32h, i'] via K=97 indicator
        # matmul, one fused num * rzb multiply, bf16 DMA out ----
        with tc.tile_pool(name="ps_norm", bufs=2, space="PSUM") as psnorm:
            for c in range(2):
                cs = slice(512 * c, 512 * (c + 1))
                rzb = psnorm.tile([128, 512], f32, tag=f"rzb{c}")
                nc.tensor.matmul(rzb[:, :], ind97, nrz[:, cs],
                                 start=True, stop=True)
                o_sb = tpool.tile([128, 512], adt, tag=f"osb{c}")
                nc.vector.scalar_tensor_tensor(
                    out=o_sb, in0=num4[:, cs],
                    scalar=1.0, in1=rzb, op0=ALU.mult, op1=ALU.mult)
                eng = nc.sync if c == 0 else nc.scalar
                eng.dma_start(out=outT[:, cs], in_=o_sb)

    nc.compile()
    return nc


def _get_nc():
    if "nc" not in _CACHE:
        _CACHE["nc"] = _build_nc()
    return _CACHE["nc"]


def make_in_maps(x, W, a_src, a_dst):
    a_ext = np.zeros((OUT_F, 2 * H), np.float32)
    for h in range(H):
        a_ext[h * HD:(h + 1) * HD, h] = a_src[h]
        a_ext[h * HD:(h + 1) * HD, H + h] = a_dst[h]
    Wa = W @ a_ext
    # ind97[k, p] = 1 iff k == 32*(p//32) (Z-row broadcast), pre-cast
    # to bf16 so the device needs no conversion
    import ml_dtypes
    ind97 = np.zeros((3 * HD + 1, OUT_F), ml_dtypes.bfloat16)
    for h in range(H):
        ind97[HD * h, HD * h:HD * (h + 1)] = 1.0
    return [
        {"xT": np.ascontiguousarray(x[c].T).astype(np.float16),
         "WWa": np.concatenate([W, Wa], axis=1).astype(np.float16),
         "ind97": ind97}
        for c in range(N_CORES)
    ]


def kernel(x, W, a_src, a_dst):
    from concourse.bass_utils import run_bass_kernel_spmd

    x = np.asarray(x, dtype=np.float32)
    W = np.asarray(W, dtype=np.float32)
    a_src = np.asarray(a_src, dtype=np.float32)
    a_dst = np.asarray(a_dst, dtype=np.float32)

    nc = _get_nc()
    in_maps = make_in_maps(x, W, a_src, a_dst)
    res = run_bass_kernel_spmd(nc, in_maps, core_ids=list(range(N_CORES)))
    out = np.stack([np.asarray(res.results[c]["outT"]).astype(np.float32).T
                    for c in range(N_CORES)], axis=0)
    return np.ascontiguousarray(out, dtype=np.float32)


# revision 6
# speedup vs baseline: 1.2125x; 1.2125x over previous
"""GAT layer kernel for Trainium2, 8-core data-parallel over batch.

Math (per batch b, head h):
    h = x @ W                              [N, H*HD]
    s_n = <h[n, h*HD:(h+1)*HD], a_src[h]>  t_n likewise with a_dst
    A[j, i] = exp(leakyrelu(s_i + t_j, 0.2))
    out[i]  = (sum_j A[j, i] * h_j) / (sum_j A[j, i])

Key identity: softmax columns are scale-invariant, so drop the e^{s_i}
column factor entirely:
    A'[j, i] = A[j, i] * e^{-s_i} = max(e^{t_j}, e^{0.2 t_j} * u_i),
    u_i = e^{-0.8 s_i}
Both numerator and Z pick up the same e^{-s_i}, which cancels in the
division. Each [128, N] attention tile is then ONE vector tensor_scalar
op (in0 = broadcast u row, two per-partition scalar columns e^{t_j} /
e^{0.2 t_j}, ops mult+max).

Per core (= one batch element):
  - xT and [W | Wa] ship as fp16 (host precomputes Wa = W @ a_ext and
    casts; |x|<6 and |Wa|<3 sit well inside f16 range): half the HBM
    bytes on the longest load, zero on-device casts, full 16-bit PE rate.
  - PE warm-up: the PE's HAM clock gate defaults to 1.2 GHz and only
    lifts to 2.4 GHz after ~3.4us of sustained ARRAY activity (K=1
    matmuls don't register -- only 1/128 rows active), so we burn the
    initial xT DMA wait on full-K junk matmuls from memset tiles into
    the ub_ps scratch PSUM. By the time real matmuls arrive the PE is
    at (or near) full clock instead of flipping warm mid-main-loop.
  - st rows = Wa^T @ xT via one small fp16 matmul per column half;
    u = Exp(-0.8 s) on ACT.
  - u broadcast [1,N] -> [128,N]: head 0 via a K=1 ones-column outer
    product on the PE (c0 copy-out on DVE, c1 on ACT); heads 1-3 via
    DRAM round-trip broadcast DMAs, split per column half and issued as
    soon as the matching uexp half lands.
  - h_node blocks AND transposed t columns from the SAME per-j-tile
    matmul with rhs [W | Wa] (136 fp16 cols; PSUM pitch bank-aligned at
    256 f32). PE order: warmups, st c0, hnst jt0-3, st c1, hnst jt4-7,
    u outer products -- projections never wait on the u chain.
  - PSUM pools: st / hnst / ub live in SEPARATE pools (pools release as
    a unit, so head 0's output banks must land on the early-releasing
    st pool, not behind the bulk weight-carve that reads hnst). ACT
    ramp order: uexp c0, e^t cols jt0-1, uexp c1, wt jt0-1 carve (head
    0's interleaved halves consume jt0+jt1 together), u_b0 c1 copy,
    e^t cols jt2-7, wt jt2-7 carve.
  - main loop per (head, j-tile): one tensor_scalar -> A' tile (bf16),
    two 512-col matmuls accumulate [h_node | ones]^T @ A' into PSUM
    [33, N] (row 32 = Z). PE ~216 ns/512 cols, DVE ~456 ns per A' tile
    -- closely matched.
  - per-head epilogue on the otherwise-idle ACT engine, pipelined with
    later heads' bulk work: Z rows -> zq (f32, 32-aligned partitions;
    dead lanes memset to 1.0 so the tail reciprocal stays finite),
    numerator rows -> num4 (frees PSUM banks; head 3's split per half
    so the tail multiply's first half isn't gated on the second).
  - tail per column half: rz = 1/Z via ONE custom-DVE op
    (reciprocal_approx_fast, ~18 correct bits -- replaces the 4-op
    int-magic Newton chain and the blocked ACT Reciprocal), K=97 f32
    indicator matmul broadcasts it to all 128 (h,d) rows, one fused
    num * rz multiply, bf16 DMA out on separate queues.
  - host transposes out^T back to node-major (and bf16 -> f32) when
    unsharding.
"""

import numpy as np

B, N, IN_F, OUT_F, H = 8, 1024, 128, 128, 4
HD = OUT_F // H  # 32
NEG = 0.2
N_CORES = 8
NT = N // 128  # 8 node tiles
CW = 256  # hnst per-j-tile column pitch (= padded rhs width)

A_DT = "bfloat16"  # dtype of the attention tiles + matmul weights
N_WARM = 6  # full-K junk matmuls (512 cols each) to lift the HAM clock gate

_CACHE = {}


def _build_nc():
    import concourse.bacc as bacc
    import concourse.tile as tile
    from concourse import mybir

    f32 = mybir.dt.float32
    f16 = mybir.dt.float16
    adt = getattr(mybir.dt, A_DT)
    AF = mybir.ActivationFunctionType
    ALU = mybir.AluOpType

    nc = bacc.Bacc("TRN2", target_bir_lowering=False, debug=False,
                   num_devices=N_CORES)

    xT = nc.declare_dram_parameter("xT", [IN_F, N], f16, isOutput=False)
    WWa_d = nc.declare_dram_parameter("WWa", [IN_F, OUT_F + 2 * H], f16,
                                      isOutput=False)
    ind97_d = nc.declare_dram_parameter("ind97", [3 * HD + 1, OUT_F], f32,
                                        isOutput=False)
    outT = nc.declare_dram_parameter("outT", [OUT_F, N], adt, isOutput=True)

    u_dram = nc.dram_tensor("u_scratch", [H, N], adt)

    with tile.TileContext(nc) as tc:
      with (
        tc.tile_pool(name="const", bufs=1) as cpool,
        tc.tile_pool(name="atile", bufs=12) as apool,
        tc.tile_pool(name="tail", bufs=1) as tpool,
      ):
        with (
            tc.tile_pool(name="ps_st", bufs=1, space="PSUM") as psst,
            tc.tile_pool(name="ps_hn", bufs=1, space="PSUM") as pshn,
            tc.tile_pool(name="ps_ub", bufs=1, space="PSUM") as psub,
        ):
            # warm-up operands first in the GpSimd FIFO -- nothing may
            # delay the junk matmuls that lift the PE clock gate
            ones1 = cpool.tile([1, 128], adt, tag="ones1")
            nc.gpsimd.memset(ones1, 1.0)
            w128 = cpool.tile([128, 128], adt, tag="w128")
            nc.gpsimd.memset(w128, 1.0)
            wsrc = cpool.tile([128, 512], adt, tag="wsrc")
            nc.gpsimd.memset(wsrc, 1.0)

            # ---- input loads; ind97 rides the Scalar queue behind the
            # xT half so the GpSimd FIFO stays free for memsets ----
            xT_sb = cpool.tile([IN_F, N], f16, tag="xT")
            WW16 = cpool.tile([IN_F, OUT_F + 2 * H], f16, tag="WW16")
            nc.sync.dma_start(out=WW16, in_=WWa_d[:])
            nc.sync.dma_start(out=xT_sb[:, 0:512], in_=xT[:, 0:512])
            nc.scalar.dma_start(out=xT_sb[:, 512:N], in_=xT[:, 512:N])
            ind97 = cpool.tile([3 * HD + 1, OUT_F], f32, tag="ind97")
            nc.scalar.dma_start(out=ind97, in_=ind97_d[:])

            War16 = WW16[:, OUT_F:OUT_F + H]

            st_ps = psst.tile([H, N], f32, tag="st")
            u_rows = cpool.tile([H, N], adt, tag="u_rows")
            hnst = pshn.tile([128, NT * CW], f32, tag="hnst")
            hnst_v = hnst[:].rearrange("p (jt c) -> p jt c", c=CW)
            ub_ps = psub.tile([128, N], f32, tag="ub0ps")
            u_b0 = cpool.tile([128, N], adt, tag="ub0")
            u_b = [u_b0]
            for h in range(1, H):
                u_b.append(cpool.tile([128, N], adt, name=f"ub{h}",
                                      tag=f"ub{h}"))

            etc = cpool.tile([128, H * NT], f32, tag="etc")
            etc02 = cpool.tile([128, H * NT], f32, tag="etc02")
            tcols = hnst_v[:, :, OUT_F + H:OUT_F + 2 * H]
            etc_v = etc[:].rearrange("p (h jt) -> p jt h", jt=NT)
            etc02_v = etc02[:].rearrange("p (h jt) -> p jt h", jt=NT)
            wt_all = cpool.tile([128, NT * 33 * H], adt, tag="wt")
            wt_v = wt_all[:].rearrange("p (jt h c) -> p jt h c", h=H, c=33)
            nc.gpsimd.memset(wt_v[:, :, :, 32:33], 1.0)
            hn_v = hnst_v[:, :, 0:OUT_F].rearrange(
                "p jt (h d) -> p jt h d", d=HD)

            # tail tensors: zq rows land on partitions 0/32/64/96; dead
            # lanes memset to 1.0 so reciprocal_approx_fast (undefined
            # at 0) stays finite under the rzb matmul's zero weights
            NP = 3 * HD + 1
            num4 = tpool.tile([128, N], f32, tag="num4")
            zq = tpool.tile([NP, N], f32, tag="zq")
            nc.gpsimd.memset(zq, 1.0)
            rz = tpool.tile([NP, N], f32, tag="rz")

            # ---- HAM warm-up: full-K junk matmuls into the ub_ps
            # scratch (overwritten later by the real outer products) ----
            for _ in range(N_WARM):
                nc.tensor.matmul(ub_ps[:, 0:512], w128, wsrc,
                                 start=True, stop=True)

            # ---- ramp, fully 512-column-chunked ----
            bcast_eng = {1: nc.sync, 2: nc.scalar, 3: nc.gpsimd}

            def half_chain(c):
                cs = slice(512 * c, 512 * (c + 1))
                nc.tensor.matmul(st_ps[:, cs], War16, xT_sb[:, cs],
                                 start=True, stop=True)
                nc.scalar.activation(out=u_rows[:, cs], in_=st_ps[:, cs],
                                     func=AF.Exp, scale=-0.8)
                for jt in (0, 1, 2, 3) if c == 0 else (4, 5, 6, 7):
                    nc.tensor.matmul(
                        hnst[:, CW * jt:CW * jt + OUT_F + 2 * H],
                        xT_sb[:, 128 * jt:128 * (jt + 1)],
                        WW16, start=True, stop=True)
                # heads 1-3 u broadcast via DRAM round trip, per half
                nc.sync.dma_start(out=u_dram[:, cs], in_=u_rows[:, cs])
                for h in range(1, H):
                    bcast_eng[h].dma_start(
                        out=u_b[h][:, cs],
                        in_=u_dram[h:h + 1, cs].to_broadcast([128, 512]))

            half_chain(0)
            # t scalar columns for jt0-1 slot between the two uexps
            nc.scalar.activation(out=etc_v[:, 0:2], in_=tcols[:, 0:2],
                                 func=AF.Exp)
            nc.scalar.activation(out=etc02_v[:, 0:2], in_=tcols[:, 0:2],
                                 func=AF.Exp, scale=NEG)
            half_chain(1)
            # weight tiles jt0-1 first: head 0's interleaved column-half
            # loop consumes jt0 AND jt1 together, so its first matmul
            # gates on this small carve, not the jt2-7 bulk one
            nc.scalar.copy(out=wt_v[:, 0:2, :, 0:32], in_=hn_v[:, 0:2])
            # head-0 u broadcast: K=1 outer product per half; c0 copy on
            # DVE (first in its FIFO), c1 copy on ACT so the DVE can
            # start head 0's first A' tiles without waiting
            nc.tensor.matmul(ub_ps[:, 0:512], ones1, u_rows[0:1, 0:512],
                             start=True, stop=True)
            nc.tensor.matmul(ub_ps[:, 512:N], ones1, u_rows[0:1, 512:N],
                             start=True, stop=True)
            nc.vector.tensor_copy(out=u_b0[:, 0:512], in_=ub_ps[:, 0:512])
            nc.scalar.copy(out=u_b0[:, 512:N], in_=ub_ps[:, 512:N])
            nc.scalar.activation(out=etc_v[:, 2:], in_=tcols[:, 2:],
                                 func=AF.Exp)
            nc.scalar.activation(out=etc02_v[:, 2:], in_=tcols[:, 2:],
                                 func=AF.Exp, scale=NEG)
            nc.scalar.copy(out=wt_v[:, 2:, :, 0:32], in_=hn_v[:, 2:])
            wts = [wt_all[:, 132 * jt:132 * (jt + 1)] for jt in range(NT)]

        # ---- main loop: one tensor_scalar + two matmuls per (h, jt) ----
        with tc.tile_pool(name="ps_main", bufs=4, space="PSUM") as psmain:
            for h in range(H):
                oh = psmain.tile([33, N], f32, tag="oh")
                if h == 0:
                    # head 0's jt0/jt1 go in column halves ordered so
                    # their c0 work streams while c1's u row is still
                    # being copied out of the outer product; the
                    # interleaved matmuls hit disjoint PSUM banks, so
                    # accumulation groups stay well-formed per half
                    ats = [apool.tile([128, N], adt, name=f"a0{k}",
                                      tag="at") for k in range(2)]
                    for c in range(2):
                        cs = slice(512 * c, 512 * (c + 1))
                        for k in range(2):
                            nc.vector.tensor_scalar(
                                out=ats[k][:, cs], in0=u_b[0][:, cs],
                                scalar1=etc02[:, k:k + 1],
                                scalar2=etc[:, k:k + 1],
                                op0=ALU.mult, op1=ALU.max)
                        for k in range(2):
                            nc.tensor.matmul(
                                oh[:, cs], wts[k][:, 0:33], ats[k][:, cs],
                                start=(k == 0), stop=False)
                    jts = range(2, NT)
                else:
                    jts = range(NT)
                for jt in jts:
                    idx = h * NT + jt
                    a_t = apool.tile([128, N], adt, tag="at")
                    nc.vector.tensor_scalar(
                        out=a_t, in0=u_b[h],
                        scalar1=etc02[:, idx:idx + 1],
                        scalar2=etc[:, idx:idx + 1],
                        op0=ALU.mult, op1=ALU.max)
                    for c in range(2):
                        nc.tensor.matmul(
                            oh[:, 512 * c:512 * (c + 1)],
                            wts[jt][:, 33 * h:33 * (h + 1)],
                            a_t[:, 512 * c:512 * (c + 1)],
                            start=(jt == 0), stop=(jt == NT - 1))
                # per-head epilogue on ACT, pipelined with later heads'
                # bulk work: Z rows to zq partition 32h per half (each
                # half's copy starts when that half's accumulation group
                # closes), numerator rows to num4. Head 3's num copy is
                # split per half so the tail's first multiply isn't
                # gated on the second half.
                for c in range(2):
                    cs = slice(512 * c, 512 * (c + 1))
                    nc.scalar.copy(out=zq[HD * h:HD * h + 1, cs],
                                   in_=oh[32:33, cs])
                if h == 3:
                    for c in range(2):
                        cs = slice(512 * c, 512 * (c + 1))
                        nc.scalar.copy(out=num4[HD * h:HD * (h + 1), cs],
                                       in_=oh[0:32, cs])
                else:
                    nc.scalar.copy(out=num4[HD * h:HD * (h + 1), :],
                                   in_=oh[0:32, :])

        # ---- tail per column half: rz = 1/Z (one custom-DVE op), K=97
        # f32 indicator matmul broadcasts rz to all 128 output rows,
        # fused num * rzb multiply, bf16 DMA out ----
        with tc.tile_pool(name="ps_norm", bufs=2, space="PSUM") as psnorm:
            rzbs = []
            for c in range(2):
                cs = slice(512 * c, 512 * (c + 1))
                nc.vector.reciprocal_approx_fast(out=rz[:, cs],
                                                 in_=zq[:, cs])
                rzb = psnorm.tile([128, 512], f32, tag=f"rzb{c}")
                nc.tensor.matmul(rzb[:, :], ind97, rz[:, cs],
                                 start=True, stop=True)
                rzbs.append(rzb)
            for c in range(2):
                cs = slice(512 * c, 512 * (c + 1))
                o_sb = tpool.tile([128, 512], adt, tag=f"osb{c}")
                nc.vector.scalar_tensor_tensor(
                    out=o_sb, in0=num4[:, cs],
                    scalar=1.0, in1=rzbs[c], op0=ALU.mult, op1=ALU.mult)
                eng = nc.sync if c == 0 else nc.scalar
                eng.dma_start(out=outT[:, cs], in_=o_sb)

    nc.compile()
    return nc


def _get_nc():
    if "nc" not in _CACHE:
        _CACHE["nc"] = _build_nc()
    return _CACHE["nc"]


def make_in_maps(x, W, a_src, a_dst):
    a_ext = np.zeros((OUT_F, 2 * H), np.float32)
    for h in range(H):
        a_ext[h * HD:(h + 1) * HD, h] = a_src[h]
        a_ext[h * HD:(h + 1) * HD, H + h] = a_dst[h]
    Wa = W @ a_ext
    # ind97[k, p] = 1 iff k == 32*(p//32) (Z-row broadcast); f32 to
    # match the f32 reciprocal rows it multiplies
    ind97 = np.zeros((3 * HD + 1, OUT_F), np.float32)
    for h in range(H):
        ind97[HD * h, HD * h:HD * (h + 1)] = 1.0
    return [
        {"xT": np.ascontiguousarray(x[c].T).astype(np.float16),
         "WWa": np.concatenate([W, Wa], axis=1).astype(np.float16),
         "ind97": ind97}
        for c in range(N_CORES)
    ]


def kernel(x, W, a_src, a_dst):
    from concourse.bass_utils import run_bass_kernel_spmd

    x = np.asarray(x, dtype=np.float32)
    W = np.asarray(W, dtype=np.float32)
    a_src = np.asarray(a_src, dtype=np.float32)
    a_dst = np.asarray(a_dst, dtype=np.float32)

    nc = _get_nc()
    in_maps = make_in_maps(x, W, a_src, a_dst)
    res = run_bass_kernel_spmd(nc, in_maps, core_ids=list(range(N_CORES)))
    out = np.stack([np.asarray(res.results[c]["outT"]).astype(np.float32).T
                    for c in range(N_CORES)], axis=0)
    return np.ascontiguousarray(out, dtype=np.float32)


# revision 7
# speedup vs baseline: 1.3216x; 1.0900x over previous
"""GAT layer kernel for Trainium2, 8-core data-parallel over batch.

Math (per batch b, head h):
    h = x @ W                              [N, H*HD]
    s_n = <h[n, h*HD:(h+1)*HD], a_src[h]>  t_n likewise with a_dst
    A[j, i] = exp(leakyrelu(s_i + t_j, 0.2))
    out[i]  = (sum_j A[j, i] * h_j) / (sum_j A[j, i])

Key identity: softmax columns are scale-invariant, so drop the e^{s_i}
column factor entirely:
    A'[j, i] = A[j, i] * e^{-s_i} = max(e^{t_j}, e^{0.2 t_j} * u_i),
    u_i = e^{-0.8 s_i}
Both numerator and Z pick up the same e^{-s_i}, which cancels in the
division. Each [128, N] attention tile is then ONE vector tensor_scalar
op (in0 = broadcast u row, two per-partition scalar columns e^{t_j} /
e^{0.2 t_j}, ops mult+max).

Per core (= one batch element):
  - xT and [W | Wa] ship as fp16 (host precomputes Wa = W @ a_ext);
    xT halves lead their queues so the first projection never waits
    behind the small weight load's descriptor.
  - PE warm-up: the HAM clock gate defaults to 1.2 GHz and lifts to
    2.4 GHz only after a FULLY-busy ~3.4us activity window; the ramp's
    own matmul stream has small bubbles that keep resetting it (the
    baseline only flipped warm mid-main-loop). Eight back-to-back
    full-K junk matmuls from memset tiles during the initial xT DMA
    wait give one contiguous >3.4us busy window, so everything
    downstream runs at full clock.
  - st rows = Wa^T @ xT per column half; u = Exp(-0.8 s) on ACT.
  - u broadcast [1,N] -> [128,N]: head 0 via a K=1 ones-column outer
    product on the PE (c0 copy-out on DVE, c1 on ACT); heads 1-3 via
    DRAM round-trip broadcast DMAs per column half on the sync/gpsimd
    queues ONLY (a broadcast descriptor on the scalar queue would
    stall ACT compute behind the DRAM write's completion).
  - h_node blocks AND transposed t columns from the SAME per-j-tile
    matmul with rhs [W | Wa]. hnst is TWO tiles (jt0-3 / jt4-7): the
    dependency tracker is conservative on rearranged slices, so with
    one tile the jt4-7 projections pick up a false WAR against the
    jt0-1 e^t reads and stall ~1us behind uexp c1.
  - weight-tile carves (PSUM->SBUF) run on the then-idle DVE, not ACT:
    the preamble PSUM pool releases only when ALL its readers are done
    (pool granularity), and the main loop's first accumulator banks
    gate on that release -- the carves and the e^t columns are the
    last readers, so they must finish as early as possible.
  - main loop per (head, j-tile): one tensor_scalar -> A' tile (bf16,
    full [128, N] -- per-op overhead makes column-half tiles ~80%
    more expensive per element), two 512-col matmuls accumulate
    [h_node | ones]^T @ A' into PSUM [33, N] (row 32 = Z). PE ~216
    ns/512 cols vs DVE ~456 ns per A' tile -- closely matched.
  - per-head epilogue on ACT, pipelined with later heads' bulk work:
    Z rows -> zq (f32, partitions 0/32/64/96; dead lanes memset to
    1.0 so the tail reciprocal stays finite), numerator rows -> num4
    (frees PSUM banks; head 3's split per half).
  - tail per column half: rz = 1/Z via ONE custom-DVE op
    (reciprocal_approx_fast, ~18 correct bits; the ACT Reciprocal is
    gated off in bass, ln+exp thrashes activation-table loads, and the
    4-op Newton chain is 3x slower), DVE cast to bf16 (an fp32
    indicator matmul runs in LOW_HIGH double-pass mode at ~5x the bf16
    cost), K=97 bf16 indicator matmul broadcasts rz to all 128 (h,d)
    rows, one fused num * rzb multiply, bf16 DMA out on separate
    queues.
  - host transposes out^T back to node-major (and bf16 -> f32) when
    unsharding.
"""

import numpy as np

B, N, IN_F, OUT_F, H = 8, 1024, 128, 128, 4
HD = OUT_F // H  # 32
NEG = 0.2
N_CORES = 8
NT = N // 128  # 8 node tiles
NTH = NT // 2  # j-tiles per hnst half-tile
CW = 256  # hnst per-j-tile column pitch (= padded rhs width)

A_DT = "bfloat16"  # dtype of the attention tiles + matmul weights
N_WARM = 8  # full-K junk matmuls (512 cols each) to lift the HAM clock gate

_CACHE = {}


def _build_nc():
    import concourse.bacc as bacc
    import concourse.tile as tile
    from concourse import mybir

    f32 = mybir.dt.float32
    f16 = mybir.dt.float16
    adt = getattr(mybir.dt, A_DT)
    AF = mybir.ActivationFunctionType
    ALU = mybir.AluOpType

    nc = bacc.Bacc("TRN2", target_bir_lowering=False, debug=False,
                   num_devices=N_CORES)

    xT = nc.declare_dram_parameter("xT", [IN_F, N], f16, isOutput=False)
    WWa_d = nc.declare_dram_parameter("WWa", [IN_F, OUT_F + 2 * H], f16,
                                      isOutput=False)
    ind97_d = nc.declare_dram_parameter("ind97", [3 * HD + 1, OUT_F], adt,
                                        isOutput=False)
    outT = nc.declare_dram_parameter("outT", [OUT_F, N], adt, isOutput=True)

    u_dram = nc.dram_tensor("u_scratch", [H, N], adt)

    with tile.TileContext(nc) as tc:
      with (
        tc.tile_pool(name="const", bufs=1) as cpool,
        tc.tile_pool(name="atile", bufs=12) as apool,
        tc.tile_pool(name="tail", bufs=1) as tpool,
        tc.tile_pool(name="ps_ub", bufs=1, space="PSUM") as psub,
      ):
        # warm-up operands lead the GpSimd FIFO -- nothing may delay
        # the junk matmuls that lift the PE clock gate
        w128 = cpool.tile([128, 128], adt, tag="w128")
        nc.gpsimd.memset(w128, 1.0)
        wsrc = cpool.tile([128, 512], adt, tag="wsrc")
        nc.gpsimd.memset(wsrc, 1.0)
        ones1 = cpool.tile([1, 128], adt, tag="ones1")
        nc.gpsimd.memset(ones1, 1.0)

        # ---- input loads: xT halves lead their queues; ind97 rides
        # the Scalar queue early (descriptor has no waits, so it can't
        # stall later ACT compute) ----
        xT_sb = cpool.tile([IN_F, N], f16, tag="xT")
        WW16 = cpool.tile([IN_F, OUT_F + 2 * H], f16, tag="WW16")
        nc.sync.dma_start(out=xT_sb[:, 0:512], in_=xT[:, 0:512])
        nc.sync.dma_start(out=WW16, in_=WWa_d[:])
        nc.scalar.dma_start(out=xT_sb[:, 512:N], in_=xT[:, 512:N])
        ind97 = cpool.tile([3 * HD + 1, OUT_F], adt, tag="ind97")
        nc.scalar.dma_start(out=ind97, in_=ind97_d[:])

        War16 = WW16[:, OUT_F:OUT_F + H]

        u_rows = cpool.tile([H, N], adt, tag="u_rows")
        ub_ps = psub.tile([128, N], f32, tag="ub0ps")
        u_b0 = cpool.tile([128, N], adt, tag="ub0")
        u_b = [u_b0]
        for h in range(1, H):
            u_b.append(cpool.tile([128, N], adt, name=f"ub{h}",
                                  tag=f"ub{h}"))

        etc = cpool.tile([128, H * NT], f32, tag="etc")
        etc02 = cpool.tile([128, H * NT], f32, tag="etc02")
        etc_v = etc[:].rearrange("p (h jt) -> p jt h", jt=NT)
        etc02_v = etc02[:].rearrange("p (h jt) -> p jt h", jt=NT)
        wt_all = cpool.tile([128, NT * 33 * H], adt, tag="wt")
        wt_v = wt_all[:].rearrange("p (jt h c) -> p jt h c", h=H, c=33)
        nc.gpsimd.memset(wt_v[:, :, :, 32:33], 1.0)

        # tail tensors: zq rows land on partitions 0/32/64/96; dead
        # lanes memset to 1.0 so reciprocal_approx_fast (undefined at
        # 0) stays finite under the rzb matmul's zero weights
        NP = 3 * HD + 1
        num4 = tpool.tile([128, N], f32, tag="num4")
        zq = tpool.tile([NP, N], f32, tag="zq")
        nc.gpsimd.memset(zq, 1.0)
        rz = tpool.tile([NP, N], f32, tag="rz")
        rzb16 = tpool.tile([NP, N], adt, tag="rzb16")

        with tc.tile_pool(name="ps_pre", bufs=1, space="PSUM") as pspre:
            st_ps = pspre.tile([H, N], f32, tag="st")
            # hnst in TWO tiles so jt4-7 writes carry no false WAR
            # against jt0-1 reads (conservative slice tracking)
            hnst = [pspre.tile([128, NTH * CW], f32, tag=f"hnst{ht}",
                               name=f"hnst{ht}") for ht in range(2)]
            hnst_v = [t[:].rearrange("p (jt c) -> p jt c", c=CW)
                      for t in hnst]
            tcols = [v[:, :, OUT_F + H:OUT_F + 2 * H] for v in hnst_v]
            hn_v = [v[:, :, 0:OUT_F].rearrange("p jt (h d) -> p jt h d",
                                               d=HD) for v in hnst_v]

            # ---- HAM warm-up: contiguous full-K junk matmuls into the
            # ub_ps scratch (overwritten later by the real outer
            # products) ----
            for _ in range(N_WARM):
                nc.tensor.matmul(ub_ps[:, 0:512], w128, wsrc,
                                 start=True, stop=True)

            # ---- ramp, fully 512-column-chunked ----
            def half_chain(c):
                cs = slice(512 * c, 512 * (c + 1))
                nc.tensor.matmul(st_ps[:, cs], War16, xT_sb[:, cs],
                                 start=True, stop=True)
                nc.scalar.activation(out=u_rows[:, cs], in_=st_ps[:, cs],
                                     func=AF.Exp, scale=-0.8)
                for jt in range(NTH * c, NTH * (c + 1)):
                    nc.tensor.matmul(
                        hnst[c][:, CW * (jt % NTH):
                                CW * (jt % NTH) + OUT_F + 2 * H],
                        xT_sb[:, 128 * jt:128 * (jt + 1)],
                        WW16, start=True, stop=True)
                # heads 1-3 u broadcast via DRAM round trip, per column
                # half, on the sync/gpsimd queues only
                nc.sync.dma_start(out=u_dram[:, cs], in_=u_rows[:, cs])
                nc.sync.dma_start(
                    out=u_b[1][:, cs],
                    in_=u_dram[1:2, cs].to_broadcast([128, 512]))
                for h in (2, 3):
                    nc.gpsimd.dma_start(
                        out=u_b[h][:, cs],
                        in_=u_dram[h:h + 1, cs].to_broadcast([128, 512]))

            half_chain(0)
            # e^t columns jt0-1 slot between the two uexps on ACT; the
            # jt0-1 weight carve runs on the then-idle DVE
            nc.scalar.activation(out=etc_v[:, 0:2], in_=tcols[0][:, 0:2],
                                 func=AF.Exp)
            nc.scalar.activation(out=etc02_v[:, 0:2], in_=tcols[0][:, 0:2],
                                 func=AF.Exp, scale=NEG)
            nc.vector.tensor_copy(out=wt_v[:, 0:2, :, 0:32],
                                  in_=hn_v[0][:, 0:2])
            half_chain(1)
            nc.scalar.activation(out=etc_v[:, 2:4], in_=tcols[0][:, 2:4],
                                 func=AF.Exp)
            nc.scalar.activation(out=etc02_v[:, 2:4], in_=tcols[0][:, 2:4],
                                 func=AF.Exp, scale=NEG)
            nc.scalar.activation(out=etc_v[:, 4:], in_=tcols[1][:],
                                 func=AF.Exp)
            nc.scalar.activation(out=etc02_v[:, 4:], in_=tcols[1][:],
                                 func=AF.Exp, scale=NEG)
            nc.vector.tensor_copy(out=wt_v[:, 2:4, :, 0:32],
                                  in_=hn_v[0][:, 2:4])
            nc.vector.tensor_copy(out=wt_v[:, 4:, :, 0:32],
                                  in_=hn_v[1][:])
            # head-0 u broadcast: K=1 outer product per half; c0
            # copy-out on DVE, c1 on ACT so the DVE reaches the first
            # A' tile sooner
            nc.tensor.matmul(ub_ps[:, 0:512], ones1, u_rows[0:1, 0:512],
                             start=True, stop=True)
            nc.tensor.matmul(ub_ps[:, 512:N], ones1, u_rows[0:1, 512:N],
                             start=True, stop=True)
            nc.vector.tensor_copy(out=u_b0[:, 0:512], in_=ub_ps[:, 0:512])
            nc.scalar.copy(out=u_b0[:, 512:N], in_=ub_ps[:, 512:N])
            wts = [wt_all[:, 132 * jt:132 * (jt + 1)] for jt in range(NT)]

        # ---- main loop: one tensor_scalar + two matmuls per (h, jt);
        # bufs=3 so head 3 reuses head 0's banks (released mid-loop by
        # head 0's epilogue copies) ----
        with tc.tile_pool(name="ps_main", bufs=3, space="PSUM") as psmain:
            for h in range(H):
                oh = psmain.tile([33, N], f32, tag="oh")
                for jt in range(NT):
                    idx = h * NT + jt
                    a_t = apool.tile([128, N], adt, tag="at")
                    nc.vector.tensor_scalar(
                        out=a_t, in0=u_b[h],
                        scalar1=etc02[:, idx:idx + 1],
                        scalar2=etc[:, idx:idx + 1],
                        op0=ALU.mult, op1=ALU.max)
                    for c in range(2):
                        nc.tensor.matmul(
                            oh[:, 512 * c:512 * (c + 1)],
                            wts[jt][:, 33 * h:33 * (h + 1)],
                            a_t[:, 512 * c:512 * (c + 1)],
                            start=(jt == 0), stop=(jt == NT - 1))
                # per-head epilogue on ACT, pipelined with later heads'
                # bulk work: Z rows to zq partition 32h per half (each
                # half's copy starts when that half's accumulation
                # group closes), numerator rows to num4. Head 3's num
                # copy is split per half so the tail's first multiply
                # isn't gated on the second half.
                for c in range(2):
                    cs = slice(512 * c, 512 * (c + 1))
                    nc.scalar.copy(out=zq[HD * h:HD * h + 1, cs],
                                   in_=oh[32:33, cs])
                if h == 3:
                    for c in range(2):
                        cs = slice(512 * c, 512 * (c + 1))
                        nc.scalar.copy(out=num4[HD * h:HD * (h + 1), cs],
                                       in_=oh[0:32, cs])
                else:
                    nc.scalar.copy(out=num4[HD * h:HD * (h + 1), :],
                                   in_=oh[0:32, :])

        # ---- tail per column half: rz = 1/Z (one custom-DVE op),
        # bf16 cast, K=97 bf16 indicator matmul broadcasts rz to all
        # 128 output rows, fused num * rzb multiply, bf16 DMA out ----
        with tc.tile_pool(name="ps_norm", bufs=2, space="PSUM") as psnorm:
            rzbs = []
            for c in range(2):
                cs = slice(512 * c, 512 * (c + 1))
                nc.vector.reciprocal_approx_fast(out=rz[:, cs],
                                                 in_=zq[:, cs])
                nc.vector.tensor_copy(out=rzb16[:, cs], in_=rz[:, cs])
                rzb = psnorm.tile([128, 512], f32, tag=f"rzb{c}")
                nc.tensor.matmul(rzb[:, :], ind97, rzb16[:, cs],
                                 start=True, stop=True)
                rzbs.append(rzb)
            for c in range(2):
                cs = slice(512 * c, 512 * (c + 1))
                o_sb = tpool.tile([128, 512], adt, tag=f"osb{c}")
                nc.vector.scalar_tensor_tensor(
                    out=o_sb, in0=num4[:, cs],
                    scalar=1.0, in1=rzbs[c], op0=ALU.mult, op1=ALU.mult)
                eng = nc.sync if c == 0 else nc.scalar
                eng.dma_start(out=outT[:, cs], in_=o_sb)

    nc.compile()
    return nc


def _get_nc():
    if "nc" not in _CACHE:
        _CACHE["nc"] = _build_nc()
    return _CACHE["nc"]


def make_in_maps(x, W, a_src, a_dst):
    a_ext = np.zeros((OUT_F, 2 * H), np.float32)
    for h in range(H):
        a_ext[h * HD:(h + 1) * HD, h] = a_src[h]
        a_ext[h * HD:(h + 1) * HD, H + h] = a_dst[h]
    Wa = W @ a_ext
    # ind97[k, p] = 1 iff k == 32*(p//32) (Z-row broadcast), pre-cast
    # to bf16 so the device needs no conversion
    import ml_dtypes
    ind97 = np.zeros((3 * HD + 1, OUT_F), ml_dtypes.bfloat16)
    for h in range(H):
        ind97[HD * h, HD * h:HD * (h + 1)] = 1.0
    return [
        {"xT": np.ascontiguousarray(x[c].T).astype(np.float16),
         "WWa": np.concatenate([W, Wa], axis=1).astype(np.float16),
         "ind97": ind97}
        for c in range(N_CORES)
    ]


def kernel(x, W, a_src, a_dst):
    from concourse.bass_utils import run_bass_kernel_spmd

    x = np.asarray(x, dtype=np.float32)
    W = np.asarray(W, dtype=np.float32)
    a_src = np.asarray(a_src, dtype=np.float32)
    a_dst = np.asarray(a_dst, dtype=np.float32)

    nc = _get_nc()
    in_maps = make_in_maps(x, W, a_src, a_dst)
    res = run_bass_kernel_spmd(nc, in_maps, core_ids=list(range(N_CORES)))
    out = np.stack([np.asarray(res.results[c]["outT"]).astype(np.float32).T
                    for c in range(N_CORES)], axis=0)
    return np.ascontiguousarray(out, dtype=np.float32)
